# revision 36
# baseline (speedup 1.0000x reference)
"""Trainium2 Bass kernel for EpiModule (epipolar sparse attention).

Full inputs -> full output. Shards B=32 frames data-parallel across 8
NeuronCores (4 frames per core); QKV/O weights replicated.

v2 design notes (per-core device program):
  - Normalized epipolar lines are precomputed on host (O(B*N) work):
    input linesN [3, 4frames, 1024] with column n = F@c_n / (||ab||+1e-6),
    pseudo-horizontal lines for the first frame of each 16-frame video.
    This removes the on-device line/norm chain entirely.
  - cfc matmuls run f32r (1 cyc/row vs 4 for f32 -- the single biggest PE
    win); cfc kept f32 for the band/decay precision.
  - whole qkv/scores/attn path in bf16 (hsT, qT, kT, v65, attention
    weights, Wo). fp8 was tried and rejected: attention weights need bf16
    range (worst softmax row max-weight is ~e^-20, far below fp8
    subnormals) and fp8 q/k pushes global rel err to ~2e-2.
  - attn@v contracts keys on the partition dim with a 65th ones-column
    producing the softmax denominator for free.
  - PSUM split matters: psA 2x[128,1024] rotates the score tiles only;
    psB 4x[128,512] serves transposes/v-proj/attn-out/normalize/out-proj.
    Merging them into one pool cost +70% device time (rotation stalls).
  - engine split: PE matmuls; ACT exp + the normalize broadcast copy;
    DVE everything psum-adjacent (gpsimd cannot access PSUM on trn2);
    DMA moves the odd-head anorm halves between partition blocks.

Softmax max-subtraction is skipped: scores = qk/8 + mask with mask <= 0 and
every query row has a mask entry >= -20 (verified on the generator
distribution), so exp() neither overflows nor denominator-underflows in fp32.
"""

import sys

if "/opt/trn_rl_repo" not in sys.path:
    sys.path.insert(0, "/opt/trn_rl_repo")

import numpy as np

B, N, C = 32, 1024, 512
HEADS, D = 8, 64
NCORES = 8
FPC = B // NCORES           # frames per core
F_MAT_SIZE = 256
PIXEL_BAND = 3.0
DECAY_ALPHA = 3.0
FRAMES_PER_VIDEO = 16

# pseudo "horizontal line" for frame 0 of each video: l_n = [0, -1, y_n]
PSEUDO_F = np.array([[0.0, 0.0, 0.0],
                     [0.0, 0.0, -1.0],
                     [0.0, 1.0, 0.0]], dtype=np.float32)


def _bf16():
    from concourse import mybir
    return mybir.dt.np(mybir.dt.bfloat16)


def make_coordsT():
    feat = int(round(N ** 0.5))          # 32
    n = np.arange(N, dtype=np.float32)
    scale = F_MAT_SIZE / feat            # 8.0
    off = (scale - 1.0) / 2.0            # 3.5
    x = scale * (n % feat) + off
    y = scale * (n // feat) + off
    return np.stack([x, y, np.ones(N, np.float32)], axis=0)  # [3, N]


def make_linesN(F_all):
    """Host: normalized epipolar lines [3, B, N] (f32), pseudo first frames."""
    F = np.array(F_all, dtype=np.float32, copy=True)
    F[::FRAMES_PER_VIDEO] = PSEUDO_F
    coords = make_coordsT()                      # [3, N]
    lines = np.einsum("bij,jn->bin", F, coords)  # [B, 3, N]
    is_first = (np.arange(F.shape[0]) % FRAMES_PER_VIDEO == 0)
    pseudo = np.stack([np.zeros(N, np.float32),
                       -np.ones(N, np.float32),
                       coords[1]], axis=0)       # [3, N]
    lines[is_first] = pseudo[None]
    abn = np.sqrt(lines[:, 0] ** 2 + lines[:, 1] ** 2) + 1e-6  # [B, N]
    lines = lines / abn[:, None, :]
    return np.ascontiguousarray(lines.transpose(1, 0, 2))      # [3, B, N]


def build_body(tc, out_aps, in_aps, attn_bufs=6, e1_bufs=3):
    """Emit the per-core program. out_aps/in_aps: dicts of DRAM APs."""
    import contextlib

    import concourse.bass as bass  # noqa: F401
    from concourse import mybir
    from concourse.masks import make_identity

    nc = tc.nc
    f32 = mybir.dt.float32
    f32r = mybir.dt.float32r
    bf16 = mybir.dt.bfloat16
    f8 = mybir.dt.float8e4
    AF = mybir.ActivationFunctionType
    OP = mybir.AluOpType
    AX = mybir.AxisListType
    DRM = mybir.MatmulPerfMode.DoubleRow

    mm = nc.tensor.matmul

    hs_d = in_aps["hs"]                      # [4, 1024, 512] f32
    ln_d = in_aps["linesN"].bitcast(mybir.dt.float32r)   # [3, 4, 1024]
    ct_d = in_aps["coordsT"].bitcast(mybir.dt.float32r)  # [3, 1024]
    wq_d = in_aps["Wq"]                      # [512, 512] bf16
    wk_d = in_aps["Wk"]                      # [512, 512] bf16
    wv_d = in_aps["Wv"]                      # [512, 512] bf16
    wor_d = in_aps["Wo_r"]                   # [128, 4, 512] bf16
    ones_r_d = in_aps["ones_row"]            # [1, 128] f32
    ones_v_d = in_aps["ones_v"]              # [128, 8, 8, 1] bf16
    out_d = out_aps["out"]                   # [4, 1024, 512] bf16

    ctx = contextlib.ExitStack()
    with ctx, nc.allow_low_precision(reason="bf16/fp8 attention path; fp32 PSUM"):
        consts = ctx.enter_context(tc.tile_pool(name="consts", bufs=1))
        big = ctx.enter_context(tc.tile_pool(name="big", bufs=1))
        at_pool = ctx.enter_context(tc.tile_pool(name="attn", bufs=max(2, attn_bufs // 2)))
        e1_pool = ctx.enter_context(tc.tile_pool(name="e1", bufs=e1_bufs))
        hs_pool = ctx.enter_context(tc.tile_pool(name="hsld", bufs=2))
        rb_pool = ctx.enter_context(tc.tile_pool(name="rb", bufs=3))
        psA = ctx.enter_context(tc.tile_pool(name="psA", bufs=2, space="PSUM"))
        psB = ctx.enter_context(tc.tile_pool(name="psB", bufs=4, space="PSUM"))

        # ---------------- constants ----------------
        wq_sb = consts.tile([128, 4, 512], bf16)
        wk_sb = consts.tile([128, 4, 512], bf16)
        wv_sb = consts.tile([128, 4, 512], bf16)
        nc.sync.dma_start(out=wq_sb, in_=wq_d.rearrange("(cc p) d -> p cc d", p=128))
        nc.sync.dma_start(out=wk_sb, in_=wk_d.rearrange("(cc p) d -> p cc d", p=128))
        nc.sync.dma_start(out=wv_sb, in_=wv_d.rearrange("(cc p) d -> p cc d", p=128))
        wor_sb = consts.tile([128, 4, 512], bf16)
        nc.sync.dma_start(out=wor_sb, in_=wor_d)
        ln_sb = consts.tile([3, 4, 1024], f32r)
        nc.sync.dma_start(out=ln_sb, in_=ln_d)
        ct_sb = consts.tile([3, 1024], f32r)
        nc.sync.dma_start(out=ct_sb, in_=ct_d)
        ident_f = consts.tile([128, 128], f32)
        make_identity(nc, ident_f)
        ones_row = consts.tile([1, 128], f32r)
        nc.sync.dma_start(out=ones_row, in_=ones_r_d.bitcast(f32r))

        # shared across frames
        cfc = big.tile([128, 8, 1024], f32)       # [m%128, mt, n]
        anorm = big.tile([128, 4, 1024], bf16)    # [(h%2)*64+d, h//2, n]

        # double-buffered per-frame workspaces: frame f uses ws[f % 2], so
        # frame f+1's prep can be emitted inside frame f's attention stream
        # (engines execute in issue order; this fills the PE/DVE gaps of the
        # ACT-gated attention phase with next-frame matmul/copy work).
        ws = []
        for wi in range(2):
            shapes = dict(
                hsT=([128, 4, 1024], bf16),   # [c%128, cc, n]
                qT=([128, 4, 1024], bf16),    # [d%128, dc, n]
                kT=([128, 4, 1024], bf16),
                v65=([128, 8, 8, 65], bf16),  # [m%128, mt, h, d|one]
                emask=([128, 8, 1024], bf16),
                bandp=([128, 8], f32),
                band2=([128, 1], f32),
                band1=([1, 1], f32r),
                bandM=([128, 1], f32),
                band_col=([128, 1], f32),
                rec=([128, 1], f32),
                negdecay=([128, 1], f32),
                dxb=([128, 1], f32),
            )
            w = {k: big.tile(shp, dt, name=f"{k}{wi}")
                 for k, (shp, dt) in shapes.items()}
            nc.sync.dma_start(out=w["v65"][:, :, :, 64:65], in_=ones_v_d)
            ws.append(w)

        def prep_mask(f, w):
            for mt in range(8):
                ps_c = psA.tile([128, 1024], f32, tag="psA")
                for s in range(2):
                    mm(ps_c[:, s * 512:(s + 1) * 512],
                       ct_sb[:, mt * 128:(mt + 1) * 128],
                       ln_sb[:, f, s * 512:(s + 1) * 512],
                       start=True, stop=True)
                # band partial from PSUM (fp32, fused abs)
                nc.vector.tensor_reduce(out=w["bandp"][:, mt:mt + 1], in_=ps_c,
                                        axis=AX.X, op=OP.max,
                                        apply_absolute_value=True)
                # cfc = |raw| via sign-bit clear (lines pre-normalized on host)
                nc.vector.tensor_scalar(
                    out=cfc[:, mt, :].bitcast(mybir.dt.int32),
                    in0=ps_c.bitcast(mybir.dt.int32),
                    scalar1=0x7FFFFFFF, scalar2=None,
                    op0=OP.bitwise_and)

            nc.vector.tensor_reduce(out=w["band2"], in_=w["bandp"],
                                    axis=AX.X, op=OP.max)
            # partition-max via PE transpose + row reduce
            ps_bt = psB.tile([128, 512], f32, tag="psB")
            nc.tensor.transpose(ps_bt[0:1, 0:128], w["band2"], ident_f)
            nc.vector.tensor_reduce(out=w["band1"].bitcast(f32),
                                    in_=ps_bt[0:1, 0:128],
                                    axis=AX.X, op=OP.max)
            ps_b = psB.tile([128, 512], f32, tag="psB")
            mm(ps_b[:, 0:1], ones_row.bitcast(f32), w["band1"].bitcast(f32),
               start=True, stop=True)
            nc.vector.tensor_copy(out=w["bandM"], in_=ps_b[:, 0:1])
            nc.vector.tensor_scalar_mul(out=w["band_col"], in0=w["bandM"],
                                        scalar1=PIXEL_BAND / (F_MAT_SIZE // 2))
            nc.vector.tensor_scalar_add(out=w["rec"], in0=w["band_col"],
                                        scalar1=1e-6)
            nc.vector.reciprocal(out=w["rec"], in_=w["rec"])
            nc.vector.tensor_scalar_mul(out=w["negdecay"], in0=w["rec"],
                                        scalar1=-DECAY_ALPHA)
            nc.vector.scalar_tensor_tensor(out=w["dxb"], in0=w["band_col"],
                                           scalar=DECAY_ALPHA, in1=w["rec"],
                                           op0=OP.mult, op1=OP.mult)
            # emask = min(exp(-decay*cfc + decay*band), 1) -> bf16
            for mt in range(8):
                nc.scalar.activation(out=w["emask"][:, mt, :], in_=cfc[:, mt, :],
                                     func=AF.Exp, scale=w["negdecay"],
                                     bias=w["dxb"])
                nc.vector.tensor_scalar(out=w["emask"][:, mt, :],
                                        in0=w["emask"][:, mt, :],
                                        scalar1=1.0, scalar2=None, op0=OP.min)

        def prep_hst(f, w):
            for nt in range(8):
                hst = hs_pool.tile([128, 512], f32)
                nc.sync.dma_start(out=hst,
                                  in_=hs_d[f, nt * 128:(nt + 1) * 128, :])
                ps_t = psB.tile([128, 512], f32, tag="psB")
                for cc in range(4):
                    nc.tensor.transpose(ps_t[:, cc * 128:(cc + 1) * 128],
                                        hst[:, cc * 128:(cc + 1) * 128],
                                        ident_f)
                nc.vector.tensor_copy(
                    out=w["hsT"][:, :, nt * 128:(nt + 1) * 128],
                    in_=ps_t.rearrange("p (cc n) -> p cc n", cc=4))

        def prep_proj(f, w):
            for dst, w_sb in ((w["qT"], wq_sb), (w["kT"], wk_sb)):
                for dc in range(4):
                    ps_q = psA.tile([128, 1024], f32, tag="psA")
                    for s in range(2):
                        for cc in range(4):
                            mm(ps_q[:, s * 512:(s + 1) * 512],
                               w_sb[:, cc, dc * 128:(dc + 1) * 128],
                               w["hsT"][:, cc, s * 512:(s + 1) * 512],
                               start=(cc == 0), stop=(cc == 3))
                    nc.vector.tensor_copy(out=dst[:, dc, :], in_=ps_q)

            for nt in range(8):
                ps_v = psB.tile([128, 512], f32, tag="psB")
                for cc in range(4):
                    mm(ps_v, w["hsT"][:, cc, nt * 128:(nt + 1) * 128],
                       wv_sb[:, cc, :],
                       start=(cc == 0), stop=(cc == 3))
                nc.vector.tensor_copy(
                    out=w["v65"][:, nt, :, 0:64],
                    in_=ps_v.rearrange("p (h d) -> p h d", h=8))

        def attn_head(w, h):
            hb = (h % 2) * 64          # partition base of this head
            hc = h // 2                # chunk index
            qT, kT, v65, emask = w["qT"], w["kT"], w["v65"], w["emask"]
            ps_av0 = psB.tile([128, 512], f32, tag="psB")
            ps_av1 = psB.tile([128, 512], f32, tag="psB")
            ps_av = [ps_av0, ps_av1]
            for mp in range(4):
                e1 = e1_pool.tile([128, 2, 1024], bf16)
                for half in range(2):
                    mt = mp * 2 + half
                    ps_s = psA.tile([128, 1024], f32, tag="psA")
                    for s in range(2):
                        mm(ps_s[:, s * 512:(s + 1) * 512],
                           kT[hb:hb + 64, hc, mt * 128:(mt + 1) * 128],
                           qT[hb:hb + 64, hc, s * 512:(s + 1) * 512],
                           start=True, stop=True)
                    # exp to SBUF so the qk PSUM frees early
                    nc.scalar.activation(out=e1[:, half, :], in_=ps_s,
                                         func=AF.Exp, scale=1.0 / 8.0)
                at = at_pool.tile([128, 2, 1024], bf16)
                nc.vector.tensor_tensor(
                    out=at, in0=emask[:, mp * 2:mp * 2 + 2, :],
                    in1=e1, op=OP.mult)
                for half in range(2):
                    mt = mp * 2 + half
                    for s in range(2):
                        mm(ps_av[s][0:65, :], v65[:, mt, h, :],
                           at[:, half, s * 512:(s + 1) * 512],
                           start=(mt == 0), stop=(mt == 7))

            for s in range(2):
                rden = rb_pool.tile([1, 512], f32r, tag="rb")
                nc.vector.reciprocal(out=rden, in_=ps_av[s][64:65, :])
                ps_r = psB.tile([64, 512], f32, tag="psB")
                mm(ps_r, ones_row[:, 0:64], rden, start=True, stop=True)
                rb = rb_pool.tile([64, 512], f32, tag="rb")
                nc.scalar.copy(out=rb, in_=ps_r)
                if h % 2 == 0:
                    nc.vector.tensor_tensor(
                        out=anorm[0:64, h // 2, s * 512:(s + 1) * 512],
                        in0=ps_av[s][0:64, :], in1=rb, op=OP.mult)
                else:
                    ntmp = rb_pool.tile([64, 512], bf16, tag="rb")
                    nc.vector.tensor_tensor(
                        out=ntmp, in0=ps_av[s][0:64, :], in1=rb,
                        op=OP.mult)
                    nc.sync.dma_start(
                        out=anorm[64:128, h // 2, s * 512:(s + 1) * 512],
                        in_=ntmp)

        def outproj(f):
            for nt in range(8):
                ps_o = psB.tile([128, 512], f32, tag="psB")
                for j in range(4):
                    mm(ps_o, anorm[:, j, nt * 128:(nt + 1) * 128],
                       wor_sb[:, j, :],
                       start=(j == 0), stop=(j == 3))
                ot = rb_pool.tile([128, 512], bf16, tag="rb")
                nc.vector.tensor_copy(out=ot, in_=ps_o)
                nc.sync.dma_start(out=out_d[f, nt * 128:(nt + 1) * 128, :],
                                  in_=ot)

        prep_mask(0, ws[0])
        prep_hst(0, ws[0])
        prep_proj(0, ws[0])
        for f in range(FPC):
            w = ws[f % 2]
            wn = ws[(f + 1) % 2]
            for h in range(8):
                attn_head(w, h)
                if f + 1 < FPC:
                    if h == 1:
                        prep_mask(f + 1, wn)
                    elif h == 3:
                        prep_hst(f + 1, wn)
                    elif h == 5:
                        prep_proj(f + 1, wn)
            outproj(f)


def make_program_io(nc):
    from concourse import mybir
    f32 = mybir.dt.float32
    bf16 = mybir.dt.bfloat16
    ins = {
        "hs": nc.dram_tensor("hs", [FPC, N, C], f32, kind="ExternalInput").ap(),
        "linesN": nc.dram_tensor("linesN", [3, FPC, N], f32,
                                 kind="ExternalInput").ap(),
        "coordsT": nc.dram_tensor("coordsT", [3, N], f32,
                                  kind="ExternalInput").ap(),
        "Wq": nc.dram_tensor("Wq", [C, C], bf16, kind="ExternalInput").ap(),
        "Wk": nc.dram_tensor("Wk", [C, C], bf16, kind="ExternalInput").ap(),
        "Wv": nc.dram_tensor("Wv", [C, C], bf16, kind="ExternalInput").ap(),
        "Wo_r": nc.dram_tensor("Wo_r", [128, 4, C], bf16,
                               kind="ExternalInput").ap(),
        "ones_row": nc.dram_tensor("ones_row", [1, 128], f32,
                                   kind="ExternalInput").ap(),
        "ones_v": nc.dram_tensor("ones_v", [128, HEADS, HEADS, 1], bf16,
                                 kind="ExternalInput").ap(),
    }
    outs = {
        "out": nc.dram_tensor("out", [FPC, N, C], bf16,
                              kind="ExternalOutput").ap(),
    }
    return ins, outs


_CACHED = None


def _build_program():
    global _CACHED
    if _CACHED is not None:
        return _CACHED
    import concourse.tile as tile
    from concourse import bacc

    nc = bacc.Bacc("TRN2", target_bir_lowering=False, debug=False,
                   num_devices=NCORES)
    ins, outs = make_program_io(nc)
    with tile.TileContext(nc) as tc:
        build_body(tc, outs, ins)
    nc.compile()
    _CACHED = nc
    return nc


def _host_weights(Wq, Wk, Wv, Wo):
    bf = _bf16()
    Wq_p = np.asarray(Wq, np.float32).astype(bf)
    Wk_p = np.asarray(Wk, np.float32).astype(bf)
    Wv_b = np.asarray(Wv, np.float32).astype(bf)
    Wo_r = np.ascontiguousarray(
        np.asarray(Wo, np.float32).reshape(4, 2, D, C).transpose(1, 2, 0, 3)
    ).reshape(128, 4, C).astype(bf)
    return Wq_p, Wk_p, Wv_b, Wo_r


def make_in_maps(hidden_states, F_mats, Wq, Wk, Wv, Wo):
    bf = _bf16()
    hs = np.asarray(hidden_states, dtype=np.float32)
    linesN = make_linesN(F_mats)                 # [3, B, N]
    Wq_p, Wk_p, Wv_b, Wo_r = _host_weights(Wq, Wk, Wv, Wo)
    in_maps = []
    for c in range(NCORES):
        fr = slice(c * FPC, (c + 1) * FPC)
        in_maps.append({
            "ones_row": np.ones((1, 128), np.float32),
            "ones_v": np.ones((128, HEADS, HEADS, 1), np.float32).astype(bf),
            "coordsT": make_coordsT(),
            "hs": np.ascontiguousarray(hs[fr]),
            "linesN": np.ascontiguousarray(linesN[:, fr]),
            "Wq": Wq_p,
            "Wk": Wk_p,
            "Wv": Wv_b,
            "Wo_r": Wo_r,
        })
    return in_maps


_RT = None

# inputs identical on every core (staged replicated, not 8x-tiled on host)
_REPLICATED = {"coordsT", "Wq", "Wk", "Wv", "Wo_r", "ones_row", "ones_v"}


def _runtime():
    """Build the program + persistent jitted sharded callable once."""
    global _RT
    if _RT is not None:
        return _RT
    import functools
    import jax
    from jax.sharding import Mesh, PartitionSpec, NamedSharding
    try:
        from jax.experimental.shard_map import shard_map
        shard_map = functools.partial(shard_map, check_rep=False)
    except ImportError:
        from jax import shard_map
        shard_map = functools.partial(shard_map, check_vma=False)
    from concourse import bass2jax as B2J
    from concourse import mybir

    nc = _build_program()
    B2J.install_neuronx_cc_hook()

    pname = nc.partition_id_tensor.name if nc.partition_id_tensor else None
    in_names, out_names, out_avals = [], [], []
    for alloc in nc.m.functions[0].allocations:
        if not isinstance(alloc, mybir.MemoryLocationSet):
            continue
        name = alloc.memorylocations[0].name
        if alloc.kind == "ExternalInput":
            if name != pname:
                in_names.append(name)
        elif alloc.kind == "ExternalOutput":
            out_names.append(name)
            out_avals.append(
                jax.core.ShapedArray(tuple(alloc.tensor_shape),
                                     mybir.dt.np(alloc.dtype)))
    all_in = in_names + out_names + ([pname] if pname else [])

    def _body(*args):
        operands = list(args)
        if pname is not None:
            operands.append(B2J.partition_id_tensor())
        outs = B2J._bass_exec_p.bind(
            *operands, out_avals=tuple(out_avals), in_names=tuple(all_in),
            out_names=tuple(out_names), lowering_input_output_aliases=(),
            sim_require_finite=True, sim_require_nnan=True, nc=nc)
        return tuple(outs)

    devices = jax.devices()[:NCORES]
    mesh = Mesh(np.asarray(devices), ("core",))
    in_specs = tuple(
        PartitionSpec() if n in _REPLICATED else PartitionSpec("core")
        for n in in_names) + (PartitionSpec("core"),) * len(out_names)
    sharded_fn = jax.jit(
        shard_map(_body, mesh=mesh, in_specs=in_specs,
                  out_specs=(PartitionSpec("core"),) * len(out_names)),
        keep_unused=True)
    sh_core = NamedSharding(mesh, PartitionSpec("core"))
    sh_repl = NamedSharding(mesh, PartitionSpec())

    # stage per-call-constant tensors once
    staged = {
        "coordsT": jax.device_put(make_coordsT(), sh_repl),
        "ones_row": jax.device_put(np.ones((1, 128), np.float32), sh_repl),
        "ones_v": jax.device_put(
            np.ones((128, HEADS, HEADS, 1), np.float32).astype(_bf16()),
            sh_repl),
    }
    out_zeros = [
        jax.device_put(np.zeros((NCORES * av.shape[0], *av.shape[1:]),
                                av.dtype), sh_core)
        for av in out_avals]

    _RT = dict(jax=jax, nc=nc, in_names=in_names, out_names=out_names,
               sharded_fn=sharded_fn, sh_core=sh_core, sh_repl=sh_repl,
               staged=staged, out_zeros=out_zeros, wcache={})
    return _RT


def _stage_weight(rt, name, arr):
    """device_put a replicated weight, cached on content fingerprint."""
    import zlib
    key = (name, arr.shape)
    fp = zlib.crc32(arr.tobytes())
    hit = rt["wcache"].get(key)
    if hit is not None and hit[0] == fp:
        return hit[1]
    dev = rt["jax"].device_put(arr, rt["sh_repl"])
    rt["wcache"][key] = (fp, dev)
    return dev


def kernel(hidden_states, F_mats, Wq, Wk, Wv, Wo):
    rt = _runtime()
    jax = rt["jax"]

    hs = np.ascontiguousarray(hidden_states, dtype=np.float32)
    linesN = make_linesN(F_mats)     # [3, B, N]
    # global layout: block c rows [3c:3c+3] = per-core linesN[:, 4c:4c+4]
    ln = np.ascontiguousarray(
        linesN.reshape(3, NCORES, FPC, N).transpose(1, 0, 2, 3)).reshape(
        NCORES * 3, FPC, N)
    Wq_p, Wk_p, Wv_b, Wo_r = _host_weights(Wq, Wk, Wv, Wo)

    vals = {
        "hs": jax.device_put(hs, rt["sh_core"]),
        "linesN": jax.device_put(ln, rt["sh_core"]),
        "Wq": _stage_weight(rt, "Wq", Wq_p),
        "Wk": _stage_weight(rt, "Wk", Wk_p),
        "Wv": _stage_weight(rt, "Wv", Wv_b),
        "Wo_r": _stage_weight(rt, "Wo_r", Wo_r),
        **rt["staged"],
    }
    args = [vals[n] for n in rt["in_names"]] + rt["out_zeros"]
    outs = rt["sharded_fn"](*args)
    arr = outs[0]
    try:
        # fetch the 8 per-core shards concurrently (one tunnel stream each)
        from concurrent.futures import ThreadPoolExecutor
        shards = sorted(arr.addressable_shards, key=lambda s: s.index)
        assert len(shards) == NCORES
        with ThreadPoolExecutor(max_workers=NCORES) as pool:
            pieces = list(pool.map(lambda s: np.asarray(s.data), shards))
        res = np.concatenate(pieces, axis=0)
    except Exception:
        res = np.asarray(arr)
    return np.ascontiguousarray(res.astype(np.float32))


if __name__ == "__main__":
    rng = np.random.default_rng(0)
    fake = {
        "hidden_states": rng.standard_normal((B, N, C), dtype=np.float32),
        "F_mats": rng.standard_normal((B, 3, 3), dtype=np.float32),
        "Wq": rng.standard_normal((C, C), dtype=np.float32) * C ** -0.5,
        "Wk": rng.standard_normal((C, C), dtype=np.float32) * C ** -0.5,
        "Wv": rng.standard_normal((C, C), dtype=np.float32) * C ** -0.5,
        "Wo": rng.standard_normal((C, C), dtype=np.float32) * C ** -0.5,
    }
    out = kernel(**fake)
    print("out", out.shape, out.dtype, np.abs(out).mean())
